# revision 1
# baseline (speedup 1.0000x reference)
"""Trainium2 Bass kernel for nn_CrossInferenceBlock (bilinear cross attention).

Computation (T=256, S=256, F=1024, A=256):
    theta = (x @ a_w + a_b).reshape(T, S, A)
    phi   = (x @ b_w + b_b).reshape(T, S, A)
    feats = (x @ g_w + g_b).reshape(T, S, F)
    attn  = einsum("tsa,tra->tsr", theta, phi)
    out   = einsum("tsr,trf->tsf", attn, feats) / (S + T)

Sharding: data-parallel over t — each of the 8 cores takes 32 contiguous
t-slices; the Linear weights are replicated.

Layout strategy (no on-chip transposes needed):
    - x arrives pre-transposed per t-slice (F on partitions).
    - thetaT/phiT are produced A-on-partitions (lhsT = a_w/b_w natural).
    - attnT[r, s] is produced r-on-partitions (lhsT = phiT, rhs = thetaT).
    - feats is produced naturally s-on-partitions (lhsT = xT slice, rhs = g_w),
      with g_b folded in via a K=1 ones-row matmul.
    - out[s, f] comes out naturally (lhsT = attnT, rhs = feats); the 1/(S+T)
      scale is folded into the attnT PSUM->SBUF copy.

Matmuls run in fp16 (fp32 PSUM accumulation): measured end-to-end rel l2
error vs the fp32 reference is ~6e-4 (bf16 would be ~5e-3).
"""

import numpy as np

import concourse.bass as bass
import concourse.bacc as bacc
import concourse.tile as tile
from concourse import mybir
from concourse.bass_utils import run_bass_kernel_spmd

T, S, F, A = 256, 256, 1024, 256
N_CORES = 8
T_LOC = T // N_CORES          # 32 t-slices per core
P = 128
KT = F // P                   # 8 contraction tiles over F
MT_A = A // P                 # 2 output tiles over A
MT_S = S // P                 # 2 tiles over s (rows of one t-slice)
NF = 512                      # matmul free-dim chunk for F-wide outputs
NC_F = F // NF                # 2 chunks
TG = 4                        # t-slices fetched per input DMA
NG = T_LOC // TG              # 8 DMA groups per core
OUT_SCALE = 1.0 / (S + T)

F16 = mybir.dt.float16
F32 = mybir.dt.float32

_COMPILED = None


def _build():
    nc = bacc.Bacc("TRN2", target_bir_lowering=False, debug=False)

    # All inputs are host-prearranged so every DMA reads per-partition
    # CONTIGUOUS runs (4-16KB), keeping HBM transfers at full rate.
    # x: (NG, P, KT, TG, S) with t = g*TG + ti, f = kt*P + p.
    x_d = nc.dram_tensor("x", [NG, P, KT, TG, S], F16, kind="ExternalInput")
    aw_d = nc.dram_tensor("aw", [P, KT, MT_A, P], F16, kind="ExternalInput")
    bw_d = nc.dram_tensor("bw", [P, KT, MT_A, P], F16, kind="ExternalInput")
    gw_d = nc.dram_tensor("gw", [P, KT, F], F16, kind="ExternalInput")
    ab_d = nc.dram_tensor("ab", [A], F32, kind="ExternalInput")
    bb_d = nc.dram_tensor("bb", [A], F32, kind="ExternalInput")
    gb_d = nc.dram_tensor("gb", [F], F32, kind="ExternalInput")
    out_d = nc.dram_tensor("out", [T_LOC, S, F], F32, kind="ExternalOutput")

    x_ap = x_d.ap()
    aw_ap = aw_d.ap()
    bw_ap = bw_d.ap()
    gw_ap = gw_d.ap()
    ab_ap = ab_d.ap().rearrange("(mt p) -> p mt", p=P)
    bb_ap = bb_d.ap().rearrange("(mt p) -> p mt", p=P)
    out_ap = out_d.ap()

    with tile.TileContext(nc) as tc:
        with (
            tc.tile_pool(name="const", bufs=1) as const,
            tc.tile_pool(name="xin", bufs=3) as xin,
            tc.tile_pool(name="proj", bufs=6) as proj,
            tc.tile_pool(name="fsb", bufs=3) as fsb,
            tc.tile_pool(name="asb", bufs=3) as asb,
            tc.tile_pool(name="osb", bufs=6) as osb,
            tc.tile_pool(name="ps_s", bufs=4, space="PSUM") as ps_s,
            tc.tile_pool(name="ps_b", bufs=4, space="PSUM") as ps_b,
        ):
            # DMA issue order matters at startup: get the operands of the
            # first t-slice's matmuls (aw/ab, x[g=0], bw/bb) in before the
            # big g_w load so the PE starts ~3us in instead of ~15us.
            xt0 = xin.tile([P, KT, TG, S], F16, tag="xt")
            nc.sync.dma_start(out=xt0[:], in_=x_ap[0])
            aw_sb = const.tile([P, KT, MT_A, P], F16)
            nc.sync.dma_start(out=aw_sb[:], in_=aw_ap)
            ab_sb = const.tile([P, MT_A], F32)
            nc.sync.dma_start(out=ab_sb[:], in_=ab_ap)
            bw_sb = const.tile([P, KT, MT_A, P], F16)
            nc.sync.dma_start(out=bw_sb[:], in_=bw_ap)
            bb_sb = const.tile([P, MT_A], F32)
            nc.sync.dma_start(out=bb_sb[:], in_=bb_ap)
            gw_sb = const.tile([P, KT, F], F16)
            nc.sync.dma_start(out=gw_sb[:], in_=gw_ap)
            gbb_sb = const.tile([P, F], F32)
            gb_bcast = bass.AP(
                tensor=gb_d.ap().tensor,
                offset=gb_d.ap().offset,
                ap=[[0, P], [1, F]],
            )
            nc.sync.dma_start(out=gbb_sb[:], in_=gb_bcast)

            for g in range(NG):
                if g == 0:
                    xt = xt0
                else:
                    xt = xin.tile([P, KT, TG, S], F16, tag="xt")
                    nc.sync.dma_start(out=xt[:], in_=x_ap[g])

                for ti in range(TG):
                    t = g * TG + ti

                    # thetaT/phiT: [A on partitions, s free], + bias, -> fp16
                    thetaT = proj.tile([P, MT_A, S], F16, tag="thetaT")
                    phiT = proj.tile([P, MT_A, S], F16, tag="phiT")
                    for w_sb, b_sb, dst in (
                        (aw_sb, ab_sb, thetaT),
                        (bw_sb, bb_sb, phiT),
                    ):
                        for mt in range(MT_A):
                            ps = ps_s.tile([P, S], F32, tag="ps_s")
                            for kt in range(KT):
                                nc.tensor.matmul(
                                    ps[:],
                                    lhsT=w_sb[:, kt, mt, :],
                                    rhs=xt[:, kt, ti, :],
                                    start=(kt == 0),
                                    stop=(kt == KT - 1),
                                )
                            nc.vector.tensor_scalar_add(
                                dst[:, mt, :], ps[:], b_sb[:, mt : mt + 1]
                            )

                    # attnT[r, s] = sum_a phi[r, a] theta[s, a]; scale folded in
                    attnT = asb.tile([P, MT_S, S], F16, tag="attnT")
                    for rt in range(MT_S):
                        ps = ps_s.tile([P, S], F32, tag="ps_s")
                        for kt in range(MT_A):
                            nc.tensor.matmul(
                                ps[:],
                                lhsT=phiT[:, kt, rt * P : (rt + 1) * P],
                                rhs=thetaT[:, kt, :],
                                start=(kt == 0),
                                stop=(kt == MT_A - 1),
                            )
                        nc.scalar.activation(
                            out=attnT[:, rt, :],
                            in_=ps[:],
                            func=mybir.ActivationFunctionType.Copy,
                            scale=OUT_SCALE,
                        )

                    # feats: [s on partitions, f free]; g_b added on DVE
                    # during the PSUM->SBUF eviction (a K=1 bias matmul
                    # would cost a full N-column stream on the PE).
                    feats = fsb.tile([P, MT_S, F], F16, tag="feats")
                    for mt in range(MT_S):
                        for c in range(NC_F):
                            ps = ps_b.tile([P, NF], F32, tag="ps_b")
                            for kt in range(KT):
                                nc.tensor.matmul(
                                    ps[:],
                                    lhsT=xt[:, kt, ti, mt * P : (mt + 1) * P],
                                    rhs=gw_sb[:, kt, c * NF : (c + 1) * NF],
                                    start=(kt == 0),
                                    stop=(kt == KT - 1),
                                )
                            nc.vector.tensor_add(
                                feats[:, mt, c * NF : (c + 1) * NF],
                                ps[:],
                                gbb_sb[:, c * NF : (c + 1) * NF],
                            )

                    # out[s, f] = sum_r attnT[r, s] feats[r, f]; stores issue
                    # per 512-chunk so the last DMA starts one eviction earlier
                    for mt in range(MT_S):
                        out_sb = osb.tile([P, F], F32, tag="out_sb")
                        for c in range(NC_F):
                            ps = ps_b.tile([P, NF], F32, tag="ps_b")
                            for rt in range(MT_S):
                                nc.tensor.matmul(
                                    ps[:],
                                    lhsT=attnT[:, rt, mt * P : (mt + 1) * P],
                                    rhs=feats[:, rt, c * NF : (c + 1) * NF],
                                    start=(rt == 0),
                                    stop=(rt == MT_S - 1),
                                )
                            nc.vector.tensor_copy(
                                out_sb[:, c * NF : (c + 1) * NF], ps[:]
                            )
                            nc.sync.dma_start(
                                out=out_ap[
                                    t, mt * P : (mt + 1) * P, c * NF : (c + 1) * NF
                                ],
                                in_=out_sb[:, c * NF : (c + 1) * NF],
                            )

    nc.compile()
    return nc


def _get_compiled():
    global _COMPILED
    if _COMPILED is None:
        _COMPILED = _build()
    return _COMPILED


def _prep_inputs(inputs):
    x = np.asarray(inputs["batch_data"], dtype=np.float32)
    assert x.shape == (T * S, F), x.shape
    # (T, S, F) -> per-core (T_LOC, F, S) -> (NG, TG, KT, P, S) -> (NG, P, KT, TG, S)
    x16 = (
        x.reshape(T, S, F)
        .transpose(0, 2, 1)
        .astype(np.float16)
        .reshape(N_CORES, NG, TG, KT, P, S)
        .transpose(0, 1, 4, 3, 2, 5)
    )
    x16 = np.ascontiguousarray(x16)

    def tile_w(w, mt):  # (F, N) -> (P, KT, mt, 128)
        n = w.shape[1]
        return np.ascontiguousarray(
            w.astype(np.float16).reshape(KT, P, mt, n // mt).transpose(1, 0, 2, 3)
        )

    aw16 = tile_w(np.asarray(inputs["a_w"], np.float32), MT_A)
    bw16 = tile_w(np.asarray(inputs["b_w"], np.float32), MT_A)
    gw16 = tile_w(np.asarray(inputs["g_w"], np.float32), 1).reshape(P, KT, F)
    ab32 = np.ascontiguousarray(np.asarray(inputs["a_b"], np.float32))
    bb32 = np.ascontiguousarray(np.asarray(inputs["b_b"], np.float32))
    gb32 = np.ascontiguousarray(np.asarray(inputs["g_b"], np.float32))
    in_maps = []
    for c in range(N_CORES):
        in_maps.append(
            {
                "x": x16[c],
                "aw": aw16,
                "bw": bw16,
                "gw": gw16,
                "ab": ab32,
                "bb": bb32,
                "gb": gb32,
            }
        )
    return in_maps


def run_spmd(inputs, **kwargs):
    """Run the compiled kernel; returns (full_output, BassKernelResults)."""
    nc = _get_compiled()
    in_maps = _prep_inputs(inputs)
    res = run_bass_kernel_spmd(nc, in_maps, list(range(N_CORES)), **kwargs)
    out = np.concatenate(
        [np.asarray(res.results[c]["out"], np.float32) for c in range(N_CORES)],
        axis=0,
    )
    return out, res


def kernel(**inputs) -> np.ndarray:
    out, _ = run_spmd(inputs)
    return out



# revision 4
# speedup vs baseline: 1.0044x; 1.0044x over previous
"""Trainium2 Bass kernel for nn_CrossInferenceBlock (bilinear cross attention).

Computation (T=256, S=256, F=1024, A=256):
    theta = (x @ a_w + a_b).reshape(T, S, A)
    phi   = (x @ b_w + b_b).reshape(T, S, A)
    feats = (x @ g_w + g_b).reshape(T, S, F)
    attn  = einsum("tsa,tra->tsr", theta, phi)
    out   = einsum("tsr,trf->tsf", attn, feats) / (S + T)

Sharding: data-parallel over t — each of the 8 cores takes 32 contiguous
t-slices; the Linear weights are replicated.

v2 changes over the 432us baseline (trace-driven):
  - The baseline's first matmul fired at t=17.6us: ~8.7us of fixed BSP/DMA
    ring init plus the full xt0+aw transfer. Now a_w/b_w/x[g0]/g_w are
    DMA'd in per-kt slices, interleaved, and the group-0 theta/phi (and
    feats[t0]) matmul loops run kt-OUTER so the PE starts on slice 0.
  - theta/phi matmuls process a PAIR of t-slices per instruction (N=512
    instead of 256) halving their instruction count (NX issue overhead).
  - A burst of dummy matmuls on a memset tile starts at ~6.6us (before
    any input data arrives) so the PE_HAM clock gate is already warm
    (2.4 GHz) when the real matmuls begin.
  - Output is stored fp16 (host upcasts): halves store traffic and the
    end-of-kernel drain. Adds ~2e-4 rel error; total stays ~6e-4.

Matmuls run in fp16 (fp32 PSUM accumulation). fp8 was evaluated and
rejected: TRN e4m3 gives ~3.5% rel error on this data (tolerance 2e-2)
and DoubleRow is only ~1.5x, so no precision/speed tradeoff works.
"""

import numpy as np

import concourse.bass as bass
import concourse.bacc as bacc
import concourse.tile as tile
from concourse import mybir
from concourse.bass_utils import run_bass_kernel_spmd

T, S, F, A = 256, 256, 1024, 256
N_CORES = 8
T_LOC = T // N_CORES          # 32 t-slices per core
P = 128
KT = F // P                   # 8 contraction tiles over F
MT_A = A // P                 # 2 output tiles over A
MT_S = S // P                 # 2 tiles over s (rows of one t-slice)
NF = 512                      # matmul free-dim chunk for F-wide outputs
NC_F = F // NF                # 2 chunks
TG = 4                        # t-slices fetched per input DMA group
NG = T_LOC // TG              # 8 DMA groups per core
N_WARM = 30                   # dummy warm-up matmuls (~3.2us cold)
OUT_SCALE = 1.0 / (S + T)

F16 = mybir.dt.float16
F32 = mybir.dt.float32

_COMPILED = None


def _build():
    nc = bacc.Bacc("TRN2", target_bir_lowering=False, debug=False)

    # All inputs are host-prearranged so every DMA reads per-partition
    # CONTIGUOUS runs, keeping HBM transfers at full rate.
    # x: (NG, P, KT, TG, S) with t = g*TG + ti, f = kt*P + p.
    x_d = nc.dram_tensor("x", [NG, P, KT, TG, S], F16, kind="ExternalInput")
    aw_d = nc.dram_tensor("aw", [P, KT, MT_A, P], F16, kind="ExternalInput")
    bw_d = nc.dram_tensor("bw", [P, KT, MT_A, P], F16, kind="ExternalInput")
    gw_d = nc.dram_tensor("gw", [P, KT, F], F16, kind="ExternalInput")
    ab_d = nc.dram_tensor("ab", [A], F32, kind="ExternalInput")
    bb_d = nc.dram_tensor("bb", [A], F32, kind="ExternalInput")
    gb_d = nc.dram_tensor("gb", [F], F16, kind="ExternalInput")
    out_d = nc.dram_tensor("out", [T_LOC, S, F], F16, kind="ExternalOutput")

    x_ap = x_d.ap()
    aw_ap = aw_d.ap()
    bw_ap = bw_d.ap()
    gw_ap = gw_d.ap()
    ab_ap = ab_d.ap().rearrange("(mt p) -> p mt", p=P)
    bb_ap = bb_d.ap().rearrange("(mt p) -> p mt", p=P)
    out_ap = out_d.ap()

    with tile.TileContext(nc) as tc:
        with (
            tc.tile_pool(name="const", bufs=1) as const,
            tc.tile_pool(name="xin", bufs=3) as xin,
            tc.tile_pool(name="proj", bufs=3) as proj,
            tc.tile_pool(name="fsb", bufs=3) as fsb,
            tc.tile_pool(name="asb", bufs=3) as asb,
            tc.tile_pool(name="osb", bufs=6) as osb,
            tc.tile_pool(name="ps_a", bufs=4, space="PSUM") as ps_a,
            tc.tile_pool(name="ps_b", bufs=4, space="PSUM") as ps_b,
        ):
            # --- PE_HAM warm-up: dummy matmuls on a memset tile. These
            # depend only on engine init (~6.5us), not on any DMA, so the
            # clock gate reaches 8/8 before the first real matmul.
            warm_sb = const.tile([P, P], F16)
            nc.vector.memset(warm_sb[:], 0.0)
            warm_ps = ps_a.tile([P, NF], F32, tag="ps")
            for _ in range(N_WARM):
                nc.tensor.matmul(
                    warm_ps[:, :P], lhsT=warm_sb[:], rhs=warm_sb[:],
                    start=True, stop=True,
                )

            # --- Startup DMAs, finest-useful granularity, in consumption
            # order: biases, then per-kt (aw, bw, x[g0]) triplets so the
            # first theta/phi matmul only waits for ~400KB, then gw per-kt.
            ab_sb = const.tile([P, MT_A], F32)
            nc.sync.dma_start(out=ab_sb[:], in_=ab_ap)
            bb_sb = const.tile([P, MT_A], F32)
            nc.sync.dma_start(out=bb_sb[:], in_=bb_ap)

            aw_sb = const.tile([P, KT, MT_A, P], F16)
            bw_sb = const.tile([P, KT, MT_A, P], F16)
            xt0 = xin.tile([P, KT, TG, S], F16, tag="xt")
            for kt in range(KT):
                nc.sync.dma_start(out=aw_sb[:, kt], in_=aw_ap[:, kt])
                nc.sync.dma_start(out=bw_sb[:, kt], in_=bw_ap[:, kt])
                nc.sync.dma_start(out=xt0[:, kt], in_=x_ap[0, :, kt])

            gbb_sb = const.tile([P, F], F16)
            gb_bcast = bass.AP(
                tensor=gb_d.ap().tensor,
                offset=gb_d.ap().offset,
                ap=[[0, P], [1, F]],
            )
            nc.sync.dma_start(out=gbb_sb[:], in_=gb_bcast)
            gw_sb = const.tile([P, KT, F], F16)
            for kt in range(KT):
                nc.sync.dma_start(out=gw_sb[:, kt], in_=gw_ap[:, kt])

            def emit_theta_phi(xt, half):
                """theta/phi for t-slice pair (2*half, 2*half+1): N=512."""
                thetaT = proj.tile([P, MT_A, 2, S], F16, tag="thetaT")
                phiT = proj.tile([P, MT_A, 2, S], F16, tag="phiT")
                pss = [
                    [
                        ps_a.tile([P, NF], F32, tag="ps", name=f"ps_p{pj}m{mt}")
                        for mt in range(MT_A)
                    ]
                    for pj in range(2)
                ]
                for kt in range(KT):
                    for pj, w_sb in enumerate((aw_sb, bw_sb)):
                        for mt in range(MT_A):
                            nc.tensor.matmul(
                                pss[pj][mt][:],
                                lhsT=w_sb[:, kt, mt, :],
                                rhs=xt[:, kt, 2 * half : 2 * half + 2, :],
                                start=(kt == 0),
                                stop=(kt == KT - 1),
                            )
                for pj, (dst, b_sb) in enumerate(
                    ((thetaT, ab_sb), (phiT, bb_sb))
                ):
                    for mt in range(MT_A):
                        nc.vector.tensor_scalar_add(
                            dst[:, mt], pss[pj][mt][:], b_sb[:, mt : mt + 1]
                        )
                return thetaT, phiT

            def emit_attn(thetaT, phiT, tip):
                """attnT[r, s] for one t; scale folded into the eviction."""
                attnT = asb.tile([P, MT_S, S], F16, tag="attnT")
                for rt in range(MT_S):
                    ps = ps_a.tile([P, NF], F32, tag="ps")
                    for at in range(MT_A):
                        nc.tensor.matmul(
                            ps[:, :S],
                            lhsT=phiT[:, at, tip, rt * P : (rt + 1) * P],
                            rhs=thetaT[:, at, tip, :],
                            start=(at == 0),
                            stop=(at == MT_A - 1),
                        )
                    nc.scalar.activation(
                        out=attnT[:, rt, :],
                        in_=ps[:, :S],
                        func=mybir.ActivationFunctionType.Copy,
                        scale=OUT_SCALE,
                    )
                return attnT

            def emit_feats(xt, ti, kt_outer):
                """feats[s, f] for one t; g_b added on DVE at eviction.

                kt_outer=True consumes per-kt gw/x DMA slices as they
                arrive (startup path); kt_inner spreads the evictions
                between matmul groups (steady-state path).
                """
                feats = fsb.tile([P, MT_S, F], F16, tag="feats")
                if kt_outer:
                    psf = [
                        ps_b.tile([P, NF], F32, tag="ps", name=f"psf{i}")
                        for i in range(MT_S * NC_F)
                    ]
                    for kt in range(KT):
                        for mt in range(MT_S):
                            for c in range(NC_F):
                                nc.tensor.matmul(
                                    psf[mt * NC_F + c][:],
                                    lhsT=xt[:, kt, ti, mt * P : (mt + 1) * P],
                                    rhs=gw_sb[:, kt, c * NF : (c + 1) * NF],
                                    start=(kt == 0),
                                    stop=(kt == KT - 1),
                                )
                    for mt in range(MT_S):
                        for c in range(NC_F):
                            nc.vector.tensor_add(
                                feats[:, mt, c * NF : (c + 1) * NF],
                                psf[mt * NC_F + c][:],
                                gbb_sb[:, c * NF : (c + 1) * NF],
                            )
                else:
                    for mt in range(MT_S):
                        for c in range(NC_F):
                            ps = ps_b.tile([P, NF], F32, tag="ps")
                            for kt in range(KT):
                                nc.tensor.matmul(
                                    ps[:],
                                    lhsT=xt[:, kt, ti, mt * P : (mt + 1) * P],
                                    rhs=gw_sb[:, kt, c * NF : (c + 1) * NF],
                                    start=(kt == 0),
                                    stop=(kt == KT - 1),
                                )
                            nc.vector.tensor_add(
                                feats[:, mt, c * NF : (c + 1) * NF],
                                ps[:],
                                gbb_sb[:, c * NF : (c + 1) * NF],
                            )
                return feats

            def emit_out(t, attnT, feats):
                """out[s, f] = sum_r attnT[r, s] feats[r, f]; fp16 stores
                issue per 512-chunk so the DMA starts one eviction early."""
                for mt in range(MT_S):
                    out_sb = osb.tile([P, F], F16, tag="out_sb")
                    for c in range(NC_F):
                        ps = ps_b.tile([P, NF], F32, tag="ps")
                        for rt in range(MT_S):
                            nc.tensor.matmul(
                                ps[:],
                                lhsT=attnT[:, rt, mt * P : (mt + 1) * P],
                                rhs=feats[:, rt, c * NF : (c + 1) * NF],
                                start=(rt == 0),
                                stop=(rt == MT_S - 1),
                            )
                        nc.vector.tensor_copy(
                            out_sb[:, c * NF : (c + 1) * NF], ps[:]
                        )
                        nc.sync.dma_start(
                            out=out_ap[
                                t, mt * P : (mt + 1) * P, c * NF : (c + 1) * NF
                            ],
                            in_=out_sb[:, c * NF : (c + 1) * NF],
                        )

            for g in range(NG):
                if g == 0:
                    xt = xt0
                else:
                    xt = xin.tile([P, KT, TG, S], F16, tag="xt")
                    nc.sync.dma_start(out=xt[:], in_=x_ap[g])

                th0, ph0 = emit_theta_phi(xt, 0)
                th1, ph1 = emit_theta_phi(xt, 1)
                projs = [(th0, ph0, 0), (th0, ph0, 1), (th1, ph1, 0), (th1, ph1, 1)]

                if g == 0:
                    # Startup: feats(t0) runs kt-outer against the arriving
                    # gw slices; attn(t1) fills the PE while the feats
                    # PSUM->SBUF evictions complete.
                    a0 = emit_attn(th0, ph0, 0)
                    f0 = emit_feats(xt, 0, kt_outer=True)
                    a1 = emit_attn(th0, ph0, 1)
                    emit_out(0, a0, f0)
                    f1 = emit_feats(xt, 1, kt_outer=False)
                    emit_out(1, a1, f1)
                    rest = [(2, projs[2]), (3, projs[3])]
                else:
                    rest = list(enumerate(projs))

                for ti, (thp, php, tip) in rest:
                    t = g * TG + ti
                    at_t = emit_attn(thp, php, tip)
                    f_t = emit_feats(xt, ti, kt_outer=False)
                    emit_out(t, at_t, f_t)

    nc.compile()
    return nc


def _get_compiled():
    global _COMPILED
    if _COMPILED is None:
        _COMPILED = _build()
    return _COMPILED


def _prep_inputs(inputs):
    x = np.asarray(inputs["batch_data"], dtype=np.float32)
    assert x.shape == (T * S, F), x.shape
    # (T, S, F) -> per-core (T_LOC, F, S) -> (NG, TG, KT, P, S) -> (NG, P, KT, TG, S)
    x16 = (
        x.reshape(T, S, F)
        .transpose(0, 2, 1)
        .astype(np.float16)
        .reshape(N_CORES, NG, TG, KT, P, S)
        .transpose(0, 1, 4, 3, 2, 5)
    )
    x16 = np.ascontiguousarray(x16)

    def tile_w(w, mt):  # (F, N) -> (P, KT, mt, 128)
        n = w.shape[1]
        return np.ascontiguousarray(
            w.astype(np.float16).reshape(KT, P, mt, n // mt).transpose(1, 0, 2, 3)
        )

    aw16 = tile_w(np.asarray(inputs["a_w"], np.float32), MT_A)
    bw16 = tile_w(np.asarray(inputs["b_w"], np.float32), MT_A)
    gw16 = tile_w(np.asarray(inputs["g_w"], np.float32), 1).reshape(P, KT, F)
    ab32 = np.ascontiguousarray(np.asarray(inputs["a_b"], np.float32))
    bb32 = np.ascontiguousarray(np.asarray(inputs["b_b"], np.float32))
    gb16 = np.ascontiguousarray(np.asarray(inputs["g_b"], np.float32).astype(np.float16))
    in_maps = []
    for c in range(N_CORES):
        in_maps.append(
            {
                "x": x16[c],
                "aw": aw16,
                "bw": bw16,
                "gw": gw16,
                "ab": ab32,
                "bb": bb32,
                "gb": gb16,
            }
        )
    return in_maps


def run_spmd(inputs, **kwargs):
    """Run the compiled kernel; returns (full_output, BassKernelResults)."""
    nc = _get_compiled()
    in_maps = _prep_inputs(inputs)
    res = run_bass_kernel_spmd(nc, in_maps, list(range(N_CORES)), **kwargs)
    out = np.concatenate(
        [
            np.asarray(res.results[c]["out"]).astype(np.float32)
            for c in range(N_CORES)
        ],
        axis=0,
    )
    return out, res


def kernel(**inputs) -> np.ndarray:
    out, _ = run_spmd(inputs)
    return out
